# revision 49
# baseline (speedup 1.0000x reference)
"""Trainium2 Bass kernel for DeLanNet inverse dynamics — v3.

out = tau_m + c1 + c2 + g   where per batch element (q, v=qDot, a2=qDDot):
  L = lower-tri from two MLPs on q, H = L L^T
  w = L^T v, alpha = A^T v, A = diag(Dd v) + lower(Do v)
  tau + c2a = L (L^T a2 + alpha);  c2b = A w
  c1 = 2 D^T pu,  pu = [v*w | v_i w_j]
  g = MLP_g(q)

v3 vs v2 (HW 178us):
  - PE delivers L, LT, v, a2, vgat (outer-product gather) and the bias row
    directly: [WLd|WLdT] / [WLo|WLoT] widened tables and a 15-row
    [vT; a2T; ones] family -> no Pool transposes, no input casts, no
    scatter prep; matmul count unchanged (LDWEIGHTS-bound)
  - smalls emitted once per GROUP (not per 4-subtile window) -> ~half the
    vector-engine instruction dispatches
  - sq via tensor_scalar pow2 on DVE 4x mode (opt), placement knobs
  - backbone matmuls at N=1024 (half the LDWEIGHTS), WIN=2 double-buffered
    contraction PSUM so the PE never waits on drains
"""

import numpy as np

import concourse.bass as bass
import concourse.bacc as bacc
import concourse.mybir as mybir
import concourse.tile as tile
from concourse.bass_utils import run_bass_kernel_spmd

DOF = 7
HID = 512
B_FULL = 32768
N_CORES = 8
B_CORE = B_FULL // N_CORES  # 4096

F32 = mybir.dt.float32
BF16 = mybir.dt.bfloat16

import os
# group sizes in batch elements (sum = B_CORE); last groups small -> short tail
GROUPS = [int(t) for t in os.environ.get(
    "K_GROUPS", "1024,1024,1024,640,384").split(",")]
assert sum(GROUPS) == B_CORE
NBMAX = max(GROUPS)
NSMAX = NBMAX // 128
WIN = int(os.environ.get("K_WIN", "2"))      # contraction window (subtiles)
REPEAT = int(os.environ.get("K_REPEAT", "1"))
SQ_TS = int(os.environ.get("K_SQTS", "0"))   # sq via tensor_scalar pow2
# which sq chunks run on Pool (rest DVE)
_SQP = os.environ.get("K_SQPOOL", "1,2,4,5")
SQ_POOL_CHUNKS = set(int(c) for c in _SQP.split(",")) if _SQP else set()
PU_POOL = int(os.environ.get("K_PUPOOL", "0"))

# S column layout
C_L = 0       # L dense [i*7+j], j inner
C_LT = 49     # L^T [j*7+i], i inner
C_DD = 98     # Dd [m*7+k], k inner
C_DO = 147    # Do [n*7+k], n in cm order
C_G = 294
C_V = 301
C_A2 = 308
C_VG = 315    # vgat: v_i for cm pair n=(i,j)
NCOL = 336

_pairs_cm = [(i, j) for j in range(DOF - 1) for i in range(j + 1, DOF)]
_grp_base = [0]
for _j in range(6):
    _grp_base.append(_grp_base[-1] + (6 - _j))


def _host_constants(Wd1, bd1, Wd2, bd2, Wo1, bo1, Wo2, bo2, Wg1, bg1, Wg2, bg2):
    import ml_dtypes
    TI, TJ = np.tril_indices(DOF, -1)
    orig_idx = np.array(
        [int(np.where((TI == i) & (TJ == j))[0][0]) for (i, j) in _pairs_cm]
    )
    Wo2_cm = Wo2[:, orig_idx]
    bo2_cm = bo2[orig_idx]

    W1cat = np.concatenate([Wd1, Wo1, Wg1], axis=1).astype(np.float32)  # [7,1536]
    b1cat = np.concatenate([bd1, bo1, bg1]).astype(np.float32)          # [1536]

    # L + L^T from the d (diag) and o (lower) a-chunks
    WLdC = np.zeros((HID, 98), np.float32)
    for m in range(DOF):
        WLdC[:, m * 7 + m] = Wd2[:, m]
        WLdC[:, 49 + m * 7 + m] = Wd2[:, m]
    WLoC = np.zeros((HID, 98), np.float32)
    for n, (i, j) in enumerate(_pairs_cm):
        WLoC[:, i * 7 + j] = Wo2_cm[:, n]
        WLoC[:, 49 + j * 7 + i] = Wo2_cm[:, n]

    Gd_n = np.zeros((HID, 49), np.float32)   # negated Gd (colsum - sq@Gd trick)
    for m in range(DOF):
        for k in range(DOF):
            Gd_n[:, m * 7 + k] = -Wd1[k, :] * Wd2[:, m]
    Go_n = np.zeros((HID, 147), np.float32)
    for n in range(21):
        for k in range(DOF):
            Go_n[:, n * 7 + k] = -Wo1[k, :] * Wo2_cm[:, n]

    bias = np.zeros(NCOL, np.float32)
    for m in range(DOF):
        bias[C_L + m * 7 + m] += bd2[m]
        bias[C_LT + m * 7 + m] += bd2[m]
    for n, (i, j) in enumerate(_pairs_cm):
        bias[C_L + i * 7 + j] += bo2_cm[n]
        bias[C_LT + j * 7 + i] += bo2_cm[n]
    bias[C_DD:C_DD + 49] = -Gd_n.sum(axis=0)
    bias[C_DO:C_DO + 147] = -Go_n.sum(axis=0)
    bias[C_G:C_G + 7] = bg2

    # 15-row family: rows 0:7 v, 7:14 a2, 14 ones (bias)
    G6 = np.zeros((15, NCOL), np.float32)
    for r in range(DOF):
        G6[r, C_V + r] = 1.0
        G6[7 + r, C_A2 + r] = 1.0
    for n, (i, j) in enumerate(_pairs_cm):
        G6[i, C_VG + n] = 1.0
    G6[14, :] = bias

    def chunkmaj(M):  # [512, N] -> [128, 4, N] with [p, c, n] = M[c*128+p, n]
        N = M.shape[1]
        return M.reshape(4, 128, N).transpose(1, 0, 2).copy()

    # 2-row-packed backbone weights: even chunks at partitions 0:7, odd at
    # 32:39 -> two K=7 matmuls run concurrently in different row-groups
    W1p = np.zeros((39, 768), np.float32)
    for pj in range(6):
        W1p[0:7, pj * 128:(pj + 1) * 128] = W1cat[:, (2 * pj) * 128:(2 * pj + 1) * 128]
        W1p[32:39, pj * 128:(pj + 1) * 128] = W1cat[:, (2 * pj + 1) * 128:(2 * pj + 2) * 128]

    bf = ml_dtypes.bfloat16
    return {
        "W1p": W1p.astype(bf),                             # [39, 768] bf16
        "b1sb": b1cat.reshape(12, 128).T.copy(),           # [128, 12] f32
        "WLdC": chunkmaj(WLdC).astype(bf),                 # [128, 4, 98]
        "WLoC": chunkmaj(WLoC).astype(bf),
        "Gdn": chunkmaj(Gd_n).astype(bf),
        "Gon": chunkmaj(Go_n).astype(bf),                  # [128, 4, 147]
        "Wg2c": chunkmaj(Wg2.astype(np.float32)).astype(bf),  # [128, 4, 7]
        "G6": G6.astype(bf),                               # [15, 336]
    }


def build_bass():
    MUL = mybir.AluOpType.mult
    ADD = mybir.AluOpType.add
    POW = mybir.AluOpType.pow
    TANH = mybir.ActivationFunctionType.Tanh
    COPYF = mybir.ActivationFunctionType.Copy
    X = mybir.AxisListType.X

    nc = bacc.Bacc("TRN2", target_bir_lowering=False, debug=False)

    xqT = nc.dram_tensor("xqT", [DOF, B_CORE], BF16, kind="ExternalInput").ap()
    xvaT = nc.dram_tensor("xvaT", [15, B_CORE], BF16, kind="ExternalInput").ap()
    W1p_d = nc.dram_tensor("W1p", [39, 768], BF16, kind="ExternalInput").ap()
    b1sb_d = nc.dram_tensor("b1sb", [128, 12], F32, kind="ExternalInput").ap()
    WLdC_d = nc.dram_tensor("WLdC", [128, 4, 98], BF16, kind="ExternalInput").ap()
    WLoC_d = nc.dram_tensor("WLoC", [128, 4, 98], BF16, kind="ExternalInput").ap()
    Gdn_d = nc.dram_tensor("Gdn", [128, 4, 49], BF16, kind="ExternalInput").ap()
    Gon_d = nc.dram_tensor("Gon", [128, 4, 147], BF16, kind="ExternalInput").ap()
    Wg2_d = nc.dram_tensor("Wg2c", [128, 4, 7], BF16, kind="ExternalInput").ap()
    G6_d = nc.dram_tensor("G6", [15, NCOL], BF16, kind="ExternalInput").ap()
    out_s = nc.dram_tensor("out_s", [B_CORE, DOF], F32, kind="ExternalOutput").ap()

    g_off = [0]
    for nb in GROUPS:
        g_off.append(g_off[-1] + nb)

    with tile.TileContext(nc) as tc, nc.allow_low_precision(reason="bf16 smalls"):
        import contextlib
        ctx = contextlib.ExitStack()
        with ctx:
            consts = ctx.enter_context(tc.tile_pool(name="consts", bufs=1))
            apool = ctx.enter_context(tc.tile_pool(name="apool", bufs=26))
            sqpool = ctx.enter_context(tc.tile_pool(name="sqpool", bufs=18))
            xq_pool = ctx.enter_context(tc.tile_pool(name="xqp", bufs=2))
            xva_pool = ctx.enter_context(tc.tile_pool(name="xvap", bufs=2))
            zpool = ctx.enter_context(tc.tile_pool(name="zp", bufs=2, space="PSUM"))
            spool = ctx.enter_context(tc.tile_pool(name="sp", bufs=2, space="PSUM"))
            ssb_pool = ctx.enter_context(tc.tile_pool(name="ssbp", bufs=3))
            adn_pool = ctx.enter_context(tc.tile_pool(name="adnp", bufs=2))
            stmp = ctx.enter_context(tc.tile_pool(name="stmp", bufs=4))
            souts = ctx.enter_context(tc.tile_pool(name="souts", bufs=3))

            # ---- constants into SBUF (critical-path first) ----
            # W1p + xq gate the first backbone matmul; issue them first
            W1_sb = consts.tile([39, 768], BF16)
            nc.sync.dma_start(out=W1_sb, in_=W1p_d)
            b1_sb = consts.tile([128, 12], F32)
            nc.scalar.dma_start(out=b1_sb, in_=b1sb_d)
            # A-dense ring: memset once; smalls only ever write diag+lower,
            # so the strict-upper zeros persist across reuses
            adns = []
            for _ in range(2):
                _adn = adn_pool.tile([128, NSMAX, 49], BF16, tag="A",
                                     name="Adn")
                nc.vector.memset(_adn, 0.0)
                adns.append(_adn)
            # warm the tanh activation table before the first real act;
            # read the memset tile so this doesn't wait on the b1 DMA
            warm = consts.tile([128, 1], F32)
            nc.scalar.activation(warm, adns[0][:, 0:1, 0], TANH)
            WLdC_sb = consts.tile([128, 4, 98], BF16)
            WLoC_sb = consts.tile([128, 4, 98], BF16)
            Gdn_sb = consts.tile([128, 4, 49], BF16)
            Gon_sb = consts.tile([128, 4, 147], BF16)
            Wg2_sb = consts.tile([128, 4, 7], BF16)
            G6_sb = consts.tile([15, NCOL], BF16)

            def emit_const_dmas():
                # deferred: not read until the first contraction. On the Act
                # HWDGE queue so they overlap the input loads on sync
                nc.scalar.dma_start(out=WLdC_sb, in_=WLdC_d)
                nc.scalar.dma_start(out=WLoC_sb, in_=WLoC_d)
                nc.scalar.dma_start(out=Gdn_sb, in_=Gdn_d)
                nc.scalar.dma_start(out=Gon_sb, in_=Gon_d)
                nc.scalar.dma_start(out=Wg2_sb, in_=Wg2_d)
                nc.scalar.dma_start(out=G6_sb, in_=G6_d)

            # ---------------- phase emitters ----------------
            def emit_z_dma(g):
                b0, nb = g_off[g], GROUPS[g]
                xq_sb = xq_pool.tile([39, NBMAX], BF16, name="xq")
                nc.sync.dma_start(out=xq_sb[0:7, 0:nb], in_=xqT[:, b0 : b0 + nb])
                nc.sync.dma_start(out=xq_sb[32:39, 0:nb], in_=xqT[:, b0 : b0 + nb])
                xva_sb = xva_pool.tile([15, NBMAX], BF16, name="xva")[:, 0:nb]
                nc.sync.dma_start(out=xva_sb, in_=xvaT[:, b0 : b0 + nb])
                return xq_sb, xva_sb

            def emit_z_chunks(g, xq_sb, a_tiles, pj0, pj1):
                # pair pj -> chunks (2pj, 2pj+1) concurrently via row-groups
                nb = GROUPS[g]
                for pj in range(pj0, pj1):
                    wcols = slice(pj * 128, (pj + 1) * 128)
                    zts = [zpool.tile([128, NBMAX], F32, name="zt")[:, 0:nb]
                           for _ in range(2)]
                    for j0 in range(0, nb, 512):
                        j1 = min(j0 + 512, nb)
                        nc.tensor.matmul(
                            zts[0][:, j0:j1], lhsT=W1_sb[0:7, wcols],
                            rhs=xq_sb[0:7, j0:j1],
                            start=True, stop=True, tile_position=(0, 0),
                        )
                        nc.tensor.matmul(
                            zts[1][:, j0:j1], lhsT=W1_sb[32:39, wcols],
                            rhs=xq_sb[32:39, j0:j1],
                            start=True, stop=True, tile_position=(32, 0),
                        )
                    for o in range(2):
                        c = 2 * pj + o
                        at = apool.tile([128, NBMAX], BF16, tag="a",
                                        name="at")[:, 0:nb]
                        nc.scalar.activation(at, zts[o], TANH,
                                             bias=b1_sb[:, c : c + 1], scale=1.0)
                        a_tiles.append(at)

            def emit_sq(a_tiles, g, c0, c1):
                nb = GROUPS[g]
                sq = []
                for c in range(c0, c1):
                    st = sqpool.tile([128, NBMAX], BF16, tag="sq", name="st")[:, 0:nb]
                    if c in SQ_POOL_CHUNKS:
                        nc.gpsimd.tensor_mul(st, a_tiles[c], a_tiles[c])
                    elif SQ_TS:
                        nc.vector.tensor_scalar(st, a_tiles[c], 2.0, None, POW)
                    else:
                        nc.vector.tensor_mul(st, a_tiles[c], a_tiles[c])
                    sq.append(st)
                return sq

            def alloc_group_sbuf(g):
                ns = GROUPS[g] // 128
                Ssb = ssb_pool.tile([128, NSMAX, NCOL], BF16, tag="S", name="Ssb")[:, 0:ns]
                return Ssb

            def emit_contract_early(a_tiles, sq_tiles, xva_sb, w0, w1):
                """F6 init + L-diag + Dd families (chunks/sq 0-3)."""
                ps = spool.tile([128, WIN, 512], F32, name="ps")
                pss = [ps[:, i, 0:NCOL] for i in range(w1 - w0)]
                W = range(w0, w1)
                for i, s in enumerate(W):
                    bs = slice(s * 128, (s + 1) * 128)
                    nc.tensor.matmul(pss[i][:, :], lhsT=xva_sb[:, bs],
                                     rhs=G6_sb, start=True, stop=False)
                # c4 outer / window inner: consecutive MMs hit different
                # PSUM banks so drain-to-accumulate overlaps the next stream
                for c4 in range(4):
                    for i, s in enumerate(W):
                        bs = slice(s * 128, (s + 1) * 128)
                        nc.tensor.matmul(pss[i][:, 0:98], lhsT=a_tiles[c4][:, bs],
                                         rhs=WLdC_sb[:, c4, :],
                                         start=False, stop=False)
                for c4 in range(4):
                    for i, s in enumerate(W):
                        bs = slice(s * 128, (s + 1) * 128)
                        nc.tensor.matmul(pss[i][:, C_DD:C_DD + 49],
                                         lhsT=sq_tiles[c4][:, bs],
                                         rhs=Gdn_sb[:, c4, :],
                                         start=False, stop=False)
                return ps

            def emit_contract_late(ps, a_tiles, sq_tiles, Ssb, w0, w1,
                                   act_drain=False):
                """L-lower + Do + g families, then drain to SBUF bf16."""
                pss = [ps[:, i, 0:NCOL] for i in range(w1 - w0)]
                W = range(w0, w1)
                for c4 in range(4):
                    for i, s in enumerate(W):
                        bs = slice(s * 128, (s + 1) * 128)
                        nc.tensor.matmul(pss[i][:, 0:98], lhsT=a_tiles[4 + c4][:, bs],
                                         rhs=WLoC_sb[:, c4, :],
                                         start=False, stop=False)
                for c4 in range(4):
                    for i, s in enumerate(W):
                        bs = slice(s * 128, (s + 1) * 128)
                        nc.tensor.matmul(pss[i][:, C_DO:C_DO + 147],
                                         lhsT=sq_tiles[4 + c4][:, bs],
                                         rhs=Gon_sb[:, c4, :],
                                         start=False, stop=False)
                for c4 in range(4):
                    for i, s in enumerate(W):
                        bs = slice(s * 128, (s + 1) * 128)
                        nc.tensor.matmul(pss[i][:, C_G:C_G + 7],
                                         lhsT=a_tiles[8 + c4][:, bs],
                                         rhs=Wg2_sb[:, c4, :],
                                         start=False, stop=(c4 == 3))
                T = w1 - w0
                if act_drain:
                    nc.scalar.activation(Ssb[:, w0:w1, :], ps[:, 0:T, 0:NCOL],
                                         COPYF)
                else:
                    nc.vector.tensor_copy(Ssb[:, w0:w1, :], ps[:, 0:T, 0:NCOL])

            def smalls_ops(g, Ssb, w0, w1, tail=False):
                """Return the smalls dependency chain as a list of thunks so
                two groups' chains can be interleaved op-by-op (the engine
                FIFOs execute in emission order; a single chain is serial).
                tail=True rebalances copies/pu onto Pool (DVE saturates in
                the tail where no drains/sq compete for Pool)."""
                ops = []
                eng_cp = nc.gpsimd if tail else nc.vector
                eng_pu = nc.gpsimd if (tail or PU_POOL) else nc.vector

                def op(f):
                    ops.append(f)

                T = w1 - w0
                sl = slice(w0, w1)
                Lv = Ssb[:, sl, C_L:C_L + 49].rearrange(
                    "p t (i j) -> p t i j", j=7)
                LTv = Ssb[:, sl, C_LT:C_LT + 49].rearrange(
                    "p t (j i) -> p t j i", i=7)
                Dv = Ssb[:, sl, C_DD:C_DD + 196].rearrange(
                    "p t (r k) -> p t r k", k=7)
                gv = Ssb[:, sl, C_G:C_G + 7]
                vv = Ssb[:, sl, C_V:C_V + 7]
                a2v = Ssb[:, sl, C_A2:C_A2 + 7]
                vgv = Ssb[:, sl, C_VG:C_VG + 21]
                sh77 = (128, T, 7, 7)
                sh28 = (128, T, 28, 7)

                def tt(tag, r=7, n=7):
                    return stmp.tile([128, NSMAX, r, n], BF16,
                                     tag=tag, name=tag)[:, 0:T]

                def so(tag, n=7, dt=BF16):
                    return souts.tile([128, NSMAX, n], dt,
                                      tag=tag, name=tag)[:, 0:T]

                # dado_r = sum_k D[r,k] v_k  (A entries)
                t0 = tt("t28", 28)
                op(lambda: nc.gpsimd.tensor_mul(
                    t0, Dv, vv.unsqueeze(2).broadcast_to(sh28)))
                dado = so("dado", 28)
                op(lambda: nc.vector.reduce_sum(dado, t0, axis=X))
                dd = dado[:, :, 0:7]
                do = dado[:, :, 7:28]
                # w = L^T v ; t1a = L^T a2
                t1 = tt("t1")
                op(lambda: nc.gpsimd.tensor_mul(
                    t1, LTv, vv.unsqueeze(2).broadcast_to(sh77)))
                w_t = so("w")
                op(lambda: nc.vector.reduce_sum(w_t, t1, axis=X))
                t2 = tt("t2")
                op(lambda: nc.gpsimd.tensor_mul(
                    t2, LTv, a2v.unsqueeze(2).broadcast_to(sh77)))
                t1a = so("t1a")
                op(lambda: nc.vector.reduce_sum(t1a, t2, axis=X))
                # A dense [i*7+m] (m inner); strict upper stays zero (ring
                # buffers are memset once at startup and only diag/lower are
                # ever written)
                Adn = adn_pool.tile([128, NSMAX, 49], BF16, tag="A", name="Adn")
                diag_ap = bass.AP(
                    tensor=Adn.tensor, offset=Adn.offset,
                    ap=[Adn[:].ap[0], [49, T], [8, 7]],
                )
                op(lambda: eng_cp.tensor_copy(diag_ap, dd))
                for j in range(6):
                    nb0 = _grp_base[j]
                    cnt = 6 - j
                    low_ap = bass.AP(
                        tensor=Adn.tensor,
                        offset=Adn.offset + ((j + 1) * 7 + j),
                        ap=[Adn[:].ap[0], [49, T], [7, cnt]],
                    )
                    op(lambda low_ap=low_ap, nb0=nb0, cnt=cnt:
                       eng_cp.tensor_copy(low_ap, do[:, :, nb0:nb0 + cnt]))
                Av = Adn[:, 0:T].rearrange("p t (i m) -> p t i m", m=7)
                # alpha = A^T v (reduce over i, strided)
                t3 = tt("t3")
                op(lambda: nc.gpsimd.tensor_mul(
                    t3, Av, vv.unsqueeze(3).broadcast_to(sh77)))
                al = so("al")
                op(lambda: nc.vector.reduce_sum(
                    al, t3.rearrange("p t i m -> p t m i"), axis=X))
                # c2b = A w (reduce over m, natural)
                t4 = tt("t4")
                op(lambda: nc.gpsimd.tensor_mul(
                    t4, Av, w_t.unsqueeze(2).broadcast_to(sh77)))
                c2b = so("c2b")
                op(lambda: nc.vector.reduce_sum(c2b, t4, axis=X))
                # vec1 = t1a + alpha ; tc = L vec1
                vec1 = so("vec1")
                op(lambda: nc.vector.tensor_add(vec1, t1a, al))
                t5 = tt("t5")
                op(lambda: nc.gpsimd.tensor_mul(
                    t5, Lv, vec1.unsqueeze(2).broadcast_to(sh77)))
                tc = so("tc")
                op(lambda: nc.vector.reduce_sum(tc, t5, axis=X))
                # pu = [v*2w | v_i * 2w_j], c1 = D^T pu
                w2 = so("w2")
                op(lambda: nc.vector.tensor_scalar_mul(w2, w_t, 2.0))
                pu = so("pu", 28)
                op(lambda: eng_pu.tensor_mul(pu[:, :, 0:7], vv, w2))
                for j in range(6):
                    nb0 = 7 + _grp_base[j]
                    cnt = 6 - j
                    op(lambda j=j, nb0=nb0, cnt=cnt: eng_pu.tensor_mul(
                        pu[:, :, nb0:nb0 + cnt],
                        vgv[:, :, nb0 - 7:nb0 - 7 + cnt],
                        w2[:, :, j:j + 1].broadcast_to((128, T, cnt))))
                t6 = tt("t6", 28)
                op(lambda: nc.gpsimd.tensor_mul(
                    t6, Dv, pu.unsqueeze(3).broadcast_to(sh28)))
                c1 = so("c1")
                op(lambda: nc.vector.reduce_sum(
                    c1, t6.rearrange("p t r k -> p t k r"), axis=X))
                # out = tc + c2b + g + c1
                o3 = so("o3")
                op(lambda: nc.gpsimd.tensor_add(o3, tc, gv))
                o4 = so("o4")
                op(lambda: nc.gpsimd.tensor_add(o4, o3, c2b))
                of = so("of", 7, F32)
                op(lambda: nc.vector.tensor_add(of, o4, c1))
                b0 = g_off[g] + w0 * 128
                nbt = T * 128
                og = out_s[b0 : b0 + nbt, :].rearrange("(t p) f -> p t f", p=128)
                op(lambda: nc.sync.dma_start(out=og, in_=of))
                return ops

            def emit_smalls(g, Ssb, w0, w1):
                for f in smalls_ops(g, Ssb, w0, w1):
                    f()

            # ---------------- pipelined emission ----------------
            NG = len(GROUPS)
            for _rep in range(REPEAT):
                early_ps = {}
                pend_smalls = []
                zd = {0: emit_z_dma(0)}
                if _rep == 0:
                    emit_const_dmas()
                ats = {0: []}
                emit_z_chunks(0, zd[0][0], ats[0], 0, 6)
                sqs = {0: emit_sq(ats[0], 0, 0, 8)}
                for g in range(NG):
                    a_tiles = ats[g]
                    xva_sb = zd[g][1]
                    has_next = g + 1 < NG
                    if has_next:
                        zd[g + 1] = emit_z_dma(g + 1)
                        ats[g + 1] = []
                        sqs[g + 1] = []
                        emit_z_chunks(g + 1, zd[g + 1][0], ats[g + 1], 0, 1)
                    Ssb = alloc_group_sbuf(g)
                    ns = GROUPS[g] // 128
                    wins = [(w0, min(w0 + WIN, ns)) for w0 in range(0, ns, WIN)]
                    nw = len(wins)
                    zsplit = [1 + 5 * i // nw for i in range(nw)] + [6]
                    for i, (w0, w1) in enumerate(wins):
                        if i == 0 and g in early_ps:
                            ps = early_ps.pop(g)
                        else:
                            ps = emit_contract_early(a_tiles, sqs[g], xva_sb,
                                                     w0, w1)
                        emit_contract_late(ps, a_tiles, sqs[g], Ssb, w0, w1,
                                           act_drain=(g >= NG - 2))
                        if has_next:
                            emit_z_chunks(g + 1, zd[g + 1][0], ats[g + 1],
                                          zsplit[i], zsplit[i + 1])
                            c0 = 8 * i // nw
                            c1_ = 8 * (i + 1) // nw
                            sqs[g + 1] += emit_sq(ats[g + 1], g + 1, c0, c1_)
                        if g == NG - 1 and pend_smalls:
                            # drip the deferred chain under our contraction
                            take = max(1, len(pend_smalls) // (nw - i))
                            for f in pend_smalls[:take]:
                                f()
                            pend_smalls = pend_smalls[take:]
                    if g == NG - 2:
                        # defer: interleave with the final group's chain below
                        pend_smalls = smalls_ops(g, Ssb, 0, ns, tail=True)
                    elif g == NG - 1:
                        ops_b = smalls_ops(g, Ssb, 0, ns, tail=True)
                        import itertools
                        for pair in itertools.zip_longest(pend_smalls, ops_b):
                            for f in pair:
                                if f is not None:
                                    f()
                    else:
                        emit_smalls(g, Ssb, 0, ns)
                    if has_next:
                        # pre-emit next group's first-window early families
                        nsn = GROUPS[g + 1] // 128
                        early_ps[g + 1] = emit_contract_early(
                            ats[g + 1], sqs[g + 1], zd[g + 1][1],
                            0, min(WIN, nsn))
                    zd.pop(g); ats.pop(g)

    nc.compile()
    return nc


_CACHED = {}


def _make_in_maps(inputs):
    x = np.asarray(inputs["x"], np.float32)
    consts = _host_constants(
        *[np.asarray(inputs[k], np.float32) for k in (
            "Wd1", "bd1", "Wd2", "bd2", "Wo1", "bo1", "Wo2", "bo2",
            "Wg1", "bg1", "Wg2", "bg2")]
    )
    import ml_dtypes
    bf = ml_dtypes.bfloat16
    xqT_full = np.ascontiguousarray(x[:, 0:DOF].T).astype(bf)
    xva_full = np.empty((15, B_FULL), bf)
    xva_full[0:7] = x[:, 7:14].T
    xva_full[7:14] = x[:, 14:21].T
    xva_full[14] = 1.0
    in_maps = []
    for c in range(N_CORES):
        sl = slice(c * B_CORE, (c + 1) * B_CORE)
        m = {"xqT": np.ascontiguousarray(xqT_full[:, sl]),
             "xvaT": np.ascontiguousarray(xva_full[:, sl])}
        m.update(consts)
        in_maps.append(m)
    return in_maps


def kernel(**inputs):
    if "nc" not in _CACHED:
        _CACHED["nc"] = build_bass()
    nc = _CACHED["nc"]
    res = run_bass_kernel_spmd(nc, _make_in_maps(inputs),
                               core_ids=list(range(N_CORES)))
    outs = [res.results[c]["out_s"] for c in range(N_CORES)]
    return np.concatenate(outs, axis=0).astype(np.float32)


def profile_once(inputs, tmpdir=None):
    if "nc" not in _CACHED:
        _CACHED["nc"] = build_bass()
    nc = _CACHED["nc"]
    res = run_bass_kernel_spmd(
        nc, _make_in_maps(inputs), core_ids=list(range(N_CORES)),
        trace=True, tmpdir=tmpdir,
    )
    return res.exec_time_ns


def time_device(inputs, iters=20):
    """Min wall time of repeated dispatch with device-resident inputs."""
    import time

    import jax
    from jax.sharding import Mesh, PartitionSpec
    from jax.experimental.shard_map import shard_map
    from concourse import bass2jax
    from concourse import mybir as mb

    bass2jax.install_neuronx_cc_hook()
    if "nc" not in _CACHED:
        _CACHED["nc"] = build_bass()
    nc = _CACHED["nc"]
    in_maps = _make_in_maps(inputs)

    partition_name = (
        nc.partition_id_tensor.name if nc.partition_id_tensor else None
    )
    in_names, out_names, out_avals, zero_outs = [], [], [], []
    for alloc in nc.m.functions[0].allocations:
        if not isinstance(alloc, mb.MemoryLocationSet):
            continue
        name = alloc.memorylocations[0].name
        if alloc.kind == "ExternalInput":
            if name != partition_name:
                in_names.append(name)
        elif alloc.kind == "ExternalOutput":
            out_names.append(name)
            shape = tuple(alloc.tensor_shape)
            dtype = mb.dt.np(alloc.dtype)
            out_avals.append(jax.core.ShapedArray(shape, dtype))
            zero_outs.append(np.zeros(shape, dtype))
    n_params = len(in_names)
    all_in = list(in_names) + list(out_names)
    if partition_name is not None:
        all_in.append(partition_name)

    def _body(*args):
        operands = list(args)
        if partition_name is not None:
            operands.append(bass2jax.partition_id_tensor())
        outs = bass2jax._bass_exec_p.bind(
            *operands,
            out_avals=tuple(out_avals),
            in_names=tuple(all_in),
            out_names=tuple(out_names),
            lowering_input_output_aliases=(),
            sim_require_finite=True,
            sim_require_nnan=True,
            nc=nc,
        )
        return tuple(outs)

    devices = jax.devices()[:N_CORES]
    mesh = Mesh(np.asarray(devices), ("core",))
    nin = n_params + len(zero_outs)
    sharded = jax.jit(
        shard_map(
            _body, mesh=mesh,
            in_specs=(PartitionSpec("core"),) * nin,
            out_specs=(PartitionSpec("core"),) * len(out_names),
            check_rep=False,
        ),
    )
    concat_in = [
        np.concatenate([np.asarray(in_maps[c][nm]) for c in range(N_CORES)], axis=0)
        for nm in in_names
    ]
    concat_zeros = [
        np.zeros((N_CORES * z.shape[0], *z.shape[1:]), z.dtype) for z in zero_outs
    ]
    sharding = jax.sharding.NamedSharding(mesh, PartitionSpec("core"))
    dev_in = [jax.device_put(a, sharding) for a in concat_in + concat_zeros]
    out = sharded(*dev_in)
    jax.block_until_ready(out)
    best = float("inf")
    for _ in range(iters):
        t0 = time.perf_counter()
        out = sharded(*dev_in)
        jax.block_until_ready(out)
        best = min(best, time.perf_counter() - t0)
    return best * 1e9
